# revision 24
# baseline (speedup 1.0000x reference)
"""Trainium2 Bass kernel for a 2-layer relational GraphSAGE VGAE encoder.

Contract: kernel(**inputs) takes the FULL unsharded inputs (as produced by
setup_inputs()) and returns the full (mu, logvar) tuple.

Strategy (8 NeuronCores, SPMD single NEFF):
  - Nodes block-sharded: core c owns nodes [c*2500, (c+1)*2500), padded to 2560.
  - Edges partitioned by destination-node owner into narrow 64-dst cells so
    the segment-sum one-hot matmuls emit only 64 columns per 128-edge chunk.
  - One-hot A-matrices are built ON DEVICE per chunk with a single DVE
    tensor_scalar (iota == col) * inv — no A-value DMA at all.
  - Source-row gathers are bundled 4 chunks per indirect DMA (one [128, 4]
    offset ap) to amortize SWDGE descriptor-generation overhead.
  - Layer-1 weights, the stacked layer-2 projection weight, and biases are
    SBUF-resident (loaded once). BatchNorm (eval) is folded into layer-2
    weights on the host.
  - Per node group: SAGE-1 -> relu -> both layer-2 projections (aggregated
    side 'tab' AND self side) -> transpose tab -> per-group AllGather. No
    hrelu DRAM roundtrip.
  - Layer-2 edges are split A/B by source-group owner: A = src in groups 0-3
    (gathers depend only on the first four AllGathers and overlap the last
    group's compute + AllGather), B = src in group 4 (small tail).
"""
import sys

sys.path.insert(0, "/opt/trn_rl_repo")

import numpy as np

NCORES = 8
N = 20000
E = 100000
IN = 512
HID = 512
CAT = 2560
OUT = 256
BN_EPS = 1e-5

NLOC = N // NCORES          # 2500
NPAD = 2560                 # 20 * 128, 5 * 512
NG = NPAD // 512            # 5 node groups of 512 per core
NREL = 5
P = 128
W1 = 256                    # layer-1 dst-cell width
B1 = 512 // W1              # 2 blocks per node group
W2 = 128                    # layer-2 dst-cell width
B2 = 512 // W2              # 4 blocks per node group


# ----------------------------------------------------------------------------
# Host-side preprocessing: sharding, edge chunking, weight folding
# ----------------------------------------------------------------------------

def _chunk_edges(key, ncells, src_vals, col, val, W):
    """Group edges by per-core cell, chunk each cell into 128-edge chunks.

    key: [E'] int = core * ncells + cell   (cell < ncells)
    src_vals: [E'] int32 gather row index for each edge
    col: [E'] int in [0, W) dst position within its W-wide cell
    val: [E'] f32 one-hot value (1/cnt)

    Returns: nch [ncells] shared chunk counts (max over cores, >=1),
             base [ncells] chunk base offsets, C,
             idxT [NCORES, P, C] int32, colT/invT [NCORES, P, C] f32.
    Pad slots get col=W (matches nothing in the 0..W-1 iota -> zero row).
    """
    counts = np.bincount(key, minlength=NCORES * ncells).reshape(NCORES, ncells)
    nch = np.maximum((counts + P - 1) // P, 1).max(axis=0)  # [ncells]
    base = np.concatenate([[0], np.cumsum(nch)[:-1]])
    C = int(nch.sum())

    order = np.argsort(key, kind="stable")
    ks = key[order]
    first = np.r_[True, ks[1:] != ks[:-1]]
    run_starts = np.flatnonzero(first)
    run_id = np.cumsum(first) - 1
    pos = np.arange(len(ks)) - run_starts[run_id]

    core_s = ks // ncells
    cell_s = ks % ncells
    chunk_s = base[cell_s] + pos // P
    row_s = pos % P

    idxT = np.zeros((NCORES, P, C), np.int32)
    colT = np.full((NCORES, P, C), float(W), np.float32)
    invT = np.ones((NCORES, P, C), np.float32)
    idxT[core_s, row_s, chunk_s] = src_vals[order]
    colT[core_s, row_s, chunk_s] = col[order]
    invT[core_s, row_s, chunk_s] = val[order]
    return nch, base, C, idxT, colT, invT


def _preprocess(x, edge_index, edge_attr, Wl5, Wr5, bl5,
                Wmu_l, Wmu_r, bmu, Wlv_l, Wlv_r, blv,
                gamma, beta, run_mean, run_var):
    x = np.asarray(x, np.float32)
    src = np.asarray(edge_index[0], np.int64)
    dst = np.asarray(edge_index[1], np.int64)
    rel = np.asarray(edge_attr, np.int64)

    # --- per-node degree counts ---
    cnt1 = np.bincount(rel * N + dst, minlength=NREL * N).reshape(NREL, N)
    inv1 = 1.0 / np.maximum(cnt1, 1.0)
    cnt2 = np.bincount(dst, minlength=N)
    inv2 = 1.0 / np.maximum(cnt2, 1.0)

    core = dst // NLOC
    loc = dst % NLOC

    # layer-1 cells in DEVICE consumption order: (g, k, blk within group)
    blk1 = loc // W1
    g1 = blk1 // B1
    cell1 = g1 * (NREL * B1) + rel * B1 + (blk1 % B1)
    key1 = core * (NREL * NG * B1) + cell1
    nch1, base1, C1, i1, c1, v1 = _chunk_edges(
        key1, NREL * NG * B1, src.astype(np.int32), loc % W1,
        inv1[rel, dst], W1)

    # layer-2: gather rows from the per-src-group all-gathered tab tables.
    # ag_tab_g rows: [core][col512]. Edges split by src group so the set for
    # src group g depends only on AllGather g (deep overlap with phase 1).
    srcl = src % NLOC
    srcg = srcl // 512
    row2 = ((src // NLOC) * 512 + srcl % 512)

    blk2 = loc // W2
    key2 = core * (NG * B2) + blk2
    l2sets = []
    for sg in range(NG):
        m = srcg == sg
        l2sets.append(_chunk_edges(
            key2[m], NG * B2, row2[m].astype(np.int32), (loc % W2)[m],
            inv2[dst[m]], W2))

    # --- node features ---
    xtab = x.astype(np.float16)                           # [N, 512] gather table
    xt = np.zeros((NCORES, IN, NPAD), np.float16)         # feature-major local x
    for c in range(NCORES):
        xt[c, :, :NLOC] = x[c * NLOC:(c + 1) * NLOC].T
    # partition-major: xtP[c][p, g*2048 + kc*512 + f] = xt[c][kc*128+p, g*512+f]
    xtP = np.ascontiguousarray(
        xt.reshape(NCORES, 4, P, NG, 512).transpose(0, 2, 3, 1, 4)
        .reshape(NCORES, P, NG * 2048))

    # --- weight folding (BN eval folded into layer-2 weights) ---
    f64 = np.float64
    s = np.asarray(gamma, f64) / np.sqrt(np.asarray(run_var, f64) + BN_EPS)
    t = np.asarray(beta, f64) - np.asarray(run_mean, f64) * s

    # partition-major weightT: w[k][p, kc*512 + j] = W^T[k][kc*128+p, j]
    def _pmaj_w(W5):
        wt = np.asarray(W5, np.float32).transpose(0, 2, 1).astype(np.float16)
        wt = np.ascontiguousarray(
            wt.reshape(NREL, 4, P, HID).transpose(0, 2, 1, 3)
            .reshape(NREL, P, 4 * HID))
        # resident stacked [128, 5*2048]: col = k*2048 + kc*512 + j
        return np.ascontiguousarray(
            wt.transpose(1, 0, 2).reshape(P, NREL * 4 * HID))

    wlt5 = _pmaj_w(Wl5)
    wrt5 = _pmaj_w(Wr5)

    Wtab = np.concatenate([np.asarray(Wmu_l, f64), np.asarray(Wlv_l, f64)], 0)
    Wself = np.concatenate([np.asarray(Wmu_r, f64), np.asarray(Wlv_r, f64)], 0)
    Wall = np.concatenate([Wtab * s[None, :], Wself * s[None, :]], 0)  # [1024, 2560]
    # partition-major: wallt[p, r*1024 + j] = Wall.T[r*128+p, j]
    wallt = np.ascontiguousarray(
        Wall.T.astype(np.float16).reshape(20, P, 1024).transpose(1, 0, 2)
        .reshape(P, 20 * 1024))

    tW = (Wtab @ t).astype(np.float32)                                  # [512]
    bself = (Wself @ t + np.concatenate(
        [np.asarray(bmu, f64), np.asarray(blv, f64)])).astype(np.float32)

    # bias tiles, laid out [128, n] so a column is a per-partition scalar
    blb = np.ascontiguousarray(
        np.asarray(bl5, np.float32).reshape(NREL * 4, P).T)   # [128, 20]
    twb = np.ascontiguousarray(tW.reshape(4, P).T)            # [128, 4]
    bsb = np.ascontiguousarray(bself.reshape(4, P).T)         # [128, 4]

    iota = np.broadcast_to(
        np.arange(W1, dtype=np.float16), (P, W1)).copy()      # [128, 256]

    # concatenated chunk tables [128, C1+sum(Cg)]
    idxT = np.concatenate([i1] + [s[3] for s in l2sets], axis=2)
    colT = np.concatenate([c1] + [s[4] for s in l2sets], axis=2)
    invT = np.concatenate([v1] + [s[5] for s in l2sets], axis=2)

    meta = (tuple(nch1), tuple(base1), C1,
            tuple(tuple(s[0]) for s in l2sets),
            tuple(tuple(s[1]) for s in l2sets),
            tuple(s[2] for s in l2sets))
    in_maps = []
    for c in range(NCORES):
        in_maps.append({
            "xtab": xtab, "xt": xtP[c],
            "idx": idxT[c], "colv": colT[c], "invv": invT[c],
            "wlt5": wlt5, "wrt5": wrt5, "wallt": wallt,
            "blb": blb, "twb": twb, "bsb": bsb, "iota": iota,
        })
    return meta, in_maps


# ----------------------------------------------------------------------------
# Device kernel
# ----------------------------------------------------------------------------

def _build(meta):
    import concourse.bacc as bacc
    import concourse.bass as bass
    import concourse.tile as tile
    import concourse.mybir as mybir
    from concourse.masks import make_identity

    (nch1, base1, C1, nch2s, base2s, C2s) = meta
    nch1 = np.asarray(nch1).reshape(NG, NREL, B1)
    base1 = np.asarray(base1).reshape(NG, NREL, B1)
    nch2 = [np.asarray(n).reshape(NG, B2) for n in nch2s]
    base2 = [np.asarray(b).reshape(NG, B2) for b in base2s]
    cbase2 = np.concatenate([[0], np.cumsum(C2s)[:-1]]) + C1  # per-set offset
    CT = C1 + int(np.sum(C2s))

    f16, f32, i32 = mybir.dt.float16, mybir.dt.float32, mybir.dt.int32

    nc = bacc.Bacc("TRN2", target_bir_lowering=False, debug=False,
                   num_devices=NCORES)

    xtab_t = nc.dram_tensor("xtab", [N, IN], f16, kind="ExternalInput")
    xt_t = nc.dram_tensor("xt", [P, NG * 2048], f16, kind="ExternalInput")
    idx_t = nc.dram_tensor("idx", [P, CT], i32, kind="ExternalInput")
    col_t = nc.dram_tensor("colv", [P, CT], f32, kind="ExternalInput")
    inv_t = nc.dram_tensor("invv", [P, CT], f32, kind="ExternalInput")
    wlt5_t = nc.dram_tensor("wlt5", [P, NREL * 2048], f16, kind="ExternalInput")
    wrt5_t = nc.dram_tensor("wrt5", [P, NREL * 2048], f16, kind="ExternalInput")
    wallt_t = nc.dram_tensor("wallt", [P, 20 * 1024], f16, kind="ExternalInput")
    blb_t = nc.dram_tensor("blb", [P, NREL * 4], f32, kind="ExternalInput")
    twb_t = nc.dram_tensor("twb", [P, 4], f32, kind="ExternalInput")
    bsb_t = nc.dram_tensor("bsb", [P, 4], f32, kind="ExternalInput")
    iota_t = nc.dram_tensor("iota", [P, W1], f16, kind="ExternalInput")
    out_t = nc.dram_tensor("out", [P, NG * 2048], f32, kind="ExternalOutput")

    warm_t = nc.dram_tensor("warm", [P, 512], f16, kind="Internal")
    ag_in = nc.dram_tensor("ag_in", [NPAD, 512], f16, kind="Internal")
    ag_tab = [nc.dram_tensor(f"ag_tab{g}", [NCORES * 512, 512], f16,
                             kind="Internal", addr_space="Shared")
              for g in range(NG)]

    with tile.TileContext(nc) as tc:
        with (
            tc.tile_pool(name="constp", bufs=1) as constp,
            tc.tile_pool(name="resp", bufs=1) as resp,
            tc.tile_pool(name="iop", bufs=3) as iop,
            tc.tile_pool(name="avp", bufs=8) as avp,
            tc.tile_pool(name="actp", bufs=2) as actp,
            tc.tile_pool(name="psum", bufs=2, space="PSUM") as pp,
        ):
            # ---- constants / resident tiles ----
            ident = constp.tile([P, P], f16, name="ident", tag="ident")
            make_identity(nc, ident[:])
            blb_sb = constp.tile([P, NREL * 4], f32, name="blb_sb", tag="blb")
            nc.sync.dma_start(out=blb_sb[:], in_=blb_t.ap())
            twb_sb = constp.tile([P, 4], f32, name="twb_sb", tag="twb")
            nc.sync.dma_start(out=twb_sb[:], in_=twb_t.ap())
            bsb_sb = constp.tile([P, 4], f32, name="bsb_sb", tag="bsb")
            nc.sync.dma_start(out=bsb_sb[:], in_=bsb_t.ap())
            iota_sb = constp.tile([P, W1], f16, name="iota_sb", tag="iota")
            nc.sync.dma_start(out=iota_sb[:], in_=iota_t.ap())

            # chunk tables via gpsimd's own SWDGE queue so the first gathers
            # don't wait on sync-ring semaphore-lane chains
            idx_sb = resp.tile([P, CT], i32, name="idx_sb", tag="idx")
            nc.gpsimd.dma_start(out=idx_sb[:], in_=idx_t.ap())
            col_sb = resp.tile([P, CT], f32, name="col_sb", tag="colv")
            nc.gpsimd.dma_start(out=col_sb[:], in_=col_t.ap())
            inv_sb = resp.tile([P, CT], f32, name="inv_sb", tag="invv")
            nc.gpsimd.dma_start(out=inv_sb[:], in_=inv_t.ap())

            # resident weights
            wl5_sb = resp.tile([P, NREL * 2048], f16, name="wl5_sb", tag="wl5")
            nc.scalar.dma_start(out=wl5_sb[:], in_=wlt5_t.ap())
            wr5_sb = resp.tile([P, NREL * 2048], f16, name="wr5_sb", tag="wr5")
            nc.scalar.dma_start(out=wr5_sb[:], in_=wrt5_t.ap())
            wall_sb = resp.tile([P, 20 * 1024], f16, name="wall_sb", tag="wall")

            def wall_chunk(r, mc):
                return wall_sb[:, r * 1024 + mc * P:r * 1024 + (mc + 1) * P]

            # self-side projections + layer-2 A-partials, resident f16
            sf_sb = resp.tile([P, NG * 2048], f16, name="sf_sb", tag="sf")
            aA_sb = resp.tile([P, NG * 2048], f16, name="aA_sb", tag="aA")

            # ---- PE warm-up: a few throwaway matmuls while DMAs fill ----
            wu = constp.tile([P, 512], f16, name="wu", tag="wu")
            nc.vector.memset(wu[:], 0.0)
            wu_ps = pp.tile([P, 2048], f32, space="PSUM", name="wu_ps", tag="big")
            for i in range(32):
                nc.tensor.matmul(out=wu_ps[:, 0:512], lhsT=wu[:, 0:P],
                                 rhs=wu[:], start=(i == 0), stop=(i == 31))
            nc.vector.tensor_copy(out=wu[:], in_=wu_ps[:, 0:512])
            nc.sync.dma_start(out=warm_t.ap(), in_=wu[:])

            # ---- gather + on-device one-hot machinery --------------------
            def agg_cell(ps, b, W, nchunks, cbase, table, pref):
                """Accumulate one W-dst cell into ps[:, cc*512 + b*W ...]."""
                l2 = pref.startswith("l2")
                tiles = []
                for ci in range(nchunks):
                    gi = cbase + ci
                    gt = iop.tile([P, 512], f16, name=f"g_{pref}_{gi}",
                                  tag="gth2" if l2 else "gth",
                                  bufs=6 if l2 else 20)
                    nc.gpsimd.indirect_dma_start(
                        out=gt[:], out_offset=None,
                        in_=table.ap(),
                        in_offset=bass.IndirectOffsetOnAxis(
                            ap=idx_sb[:, gi:gi + 1], axis=0))
                    av = avp.tile([P, W1], f16, name=f"av_{pref}_{gi}",
                                  tag="av")
                    nc.vector.tensor_scalar(
                        out=av[:, 0:W], in0=iota_sb[:, 0:W],
                        scalar1=col_sb[:, gi:gi + 1],
                        scalar2=inv_sb[:, gi:gi + 1],
                        op0=mybir.AluOpType.is_equal, op1=mybir.AluOpType.mult)
                    tiles.append((gt, av))
                for cc in range(4):
                    for ci, (gt, av) in enumerate(tiles):
                        nc.tensor.matmul(
                            out=ps[:, cc * 512 + b * W:cc * 512 + (b + 1) * W],
                            lhsT=gt[:, cc * P:(cc + 1) * P],
                            rhs=av[:, 0:W],
                            start=(ci == 0), stop=(ci == nchunks - 1))

            # ====== Phase 1, g-outer: SAGE layer 1 -> both projections
            # -> transpose -> per-group AllGather (collectives overlap P1).
            def emit_ag(g):
                nc.gpsimd.collective_compute(
                    "AllGather", mybir.AluOpType.bypass,
                    replica_groups=[list(range(NCORES))],
                    ins=[ag_in.ap()[g * 512:(g + 1) * 512, :]],
                    outs=[ag_tab[g].ap()])

            def l2_block(sg):
                """Aggregate layer-2 messages whose src is in node group sg.

                sg 0-3 accumulate into the resident aA_sb partial; sg 4 adds
                the partial + self side and writes the final output.
                """
                for dg in range(NG):
                    ps = pp.tile([P, 2048], f32, space="PSUM",
                                 name=f"m2_{sg}_{dg}", tag="big")
                    for b in range(B2):
                        agg_cell(ps, b, W2, int(nch2[sg][dg, b]),
                                 int(cbase2[sg]) + int(base2[sg][dg, b]),
                                 ag_tab[sg], f"l2{sg}")
                    osl = slice(dg * 2048, (dg + 1) * 2048)
                    if sg == 0:
                        nc.vector.tensor_copy(out=aA_sb[:, osl], in_=ps[:])
                    elif sg < NG - 1:
                        nc.vector.tensor_tensor(
                            out=aA_sb[:, osl], in0=ps[:], in1=aA_sb[:, osl],
                            op=mybir.AluOpType.add)
                    else:
                        ob = actp.tile([P, 2048], f32, name=f"out_{dg}",
                                       tag="outsb", bufs=1)
                        for mc in range(4):
                            sl = slice(mc * 512, (mc + 1) * 512)
                            gsl = slice(dg * 2048 + mc * 512,
                                        dg * 2048 + (mc + 1) * 512)
                            nc.vector.tensor_tensor(
                                out=ob[:, sl], in0=ps[:, sl],
                                in1=aA_sb[:, gsl], op=mybir.AluOpType.add)
                            nc.vector.tensor_tensor(
                                out=ob[:, sl], in0=ob[:, sl],
                                in1=sf_sb[:, gsl], op=mybir.AluOpType.add)
                        nc.scalar.dma_start(
                            out=out_t.ap()[:, dg * 2048:(dg + 1) * 2048],
                            in_=ob[:])

            pending_ag = None
            for gg in range(NG):
                xg = actp.tile([P, 2048], f16, name=f"xg_{gg}", tag="xg",
                               bufs=1)
                nc.scalar.dma_start(
                    out=xg[:], in_=xt_t.ap()[:, gg * 2048:(gg + 1) * 2048])
                if gg == 0:
                    nc.scalar.dma_start(out=wall_sb[:], in_=wallt_t.ap())
                rts = []
                for k in range(NREL):
                    # --- aggregation: mean_k^T for this node group ---
                    mean_ps = pp.tile([P, 2048], f32, space="PSUM",
                                      name=f"agg_{gg}_{k}", tag="big")
                    for b in range(B1):
                        agg_cell(mean_ps, b, W1, int(nch1[gg, k, b]),
                                 int(base1[gg, k, b]), xtab_t, "l1")
                    mean_sb = []
                    for cc in range(4):
                        m = actp.tile([P, 512], f16, name=f"mean_{gg}_{k}_{cc}",
                                      tag=f"mean{cc}")
                        nc.vector.tensor_copy(
                            out=m[:], in_=mean_ps[:, cc * 512:(cc + 1) * 512])
                        mean_sb.append(m)

                    # --- dense: h = relu(Wl@mean + Wr@x + b) ---
                    h_ps = pp.tile([P, 2048], f32, space="PSUM",
                                   name=f"h_{gg}_{k}", tag="big")
                    for kc in range(4):
                        base = k * 2048 + kc * 512
                        for mc in range(4):
                            nc.tensor.matmul(
                                out=h_ps[:, mc * 512:(mc + 1) * 512],
                                lhsT=wl5_sb[:, base + mc * P:base + (mc + 1) * P],
                                rhs=mean_sb[kc][:], start=(kc == 0), stop=False)
                    for kc in range(4):
                        base = k * 2048 + kc * 512
                        for mc in range(4):
                            nc.tensor.matmul(
                                out=h_ps[:, mc * 512:(mc + 1) * 512],
                                lhsT=wr5_sb[:, base + mc * P:base + (mc + 1) * P],
                                rhs=xg[:, kc * 512:(kc + 1) * 512],
                                start=False, stop=(kc == 3))
                    rt = actp.tile([P, 2048], f16, name=f"relu_{gg}_{k}",
                                   tag=f"rt{k}", bufs=1)
                    for mc in range(4):
                        nc.vector.tensor_scalar(
                            out=rt[:, mc * 512:(mc + 1) * 512],
                            in0=h_ps[:, mc * 512:(mc + 1) * 512],
                            scalar1=blb_sb[:, k * 4 + mc:k * 4 + mc + 1],
                            scalar2=0.0,
                            op0=mybir.AluOpType.add, op1=mybir.AluOpType.max)
                    rts.append(rt)

                if pending_ag is not None:
                    emit_ag(pending_ag)
                    pending_ag = None

                # --- both layer-2 projections straight from the relu tiles
                tab_sb = []
                for half in range(2):
                    p_ps = pp.tile([P, 2048], f32, space="PSUM",
                                   name=f"proj_{gg}_{half}", tag="big")
                    for mc4 in range(4):
                        mc = half * 4 + mc4
                        o = p_ps[:, mc4 * 512:(mc4 + 1) * 512]
                        for r in range(20):
                            nc.tensor.matmul(
                                out=o, lhsT=wall_chunk(r, mc),
                                rhs=rts[r // 4][:, (r % 4) * 512:(r % 4 + 1) * 512],
                                start=(r == 0), stop=(r == 19))
                    if half == 0:
                        for mc4 in range(4):
                            tab = actp.tile([P, 512], f16, name=f"tab_{gg}_{mc4}",
                                            tag=f"tab{mc4}", bufs=1)
                            nc.vector.tensor_scalar(
                                out=tab[:], in0=p_ps[:, mc4 * 512:(mc4 + 1) * 512],
                                scalar1=twb_sb[:, mc4:mc4 + 1],
                                scalar2=None, op0=mybir.AluOpType.add)
                            tab_sb.append(tab)
                    else:
                        for mc4 in range(4):
                            nc.vector.tensor_scalar(
                                out=sf_sb[:, gg * 2048 + mc4 * 512:
                                          gg * 2048 + (mc4 + 1) * 512],
                                in0=p_ps[:, mc4 * 512:(mc4 + 1) * 512],
                                scalar1=bsb_sb[:, mc4:mc4 + 1],
                                scalar2=None, op0=mybir.AluOpType.add)

                tr_ps = pp.tile([P, 2048], f16, space="PSUM",
                                name=f"tr_{gg}", tag="big")
                for ns in range(4):
                    for mc in range(4):
                        nc.tensor.transpose(
                            out=tr_ps[:, (ns * 4 + mc) * P:(ns * 4 + mc + 1) * P],
                            in_=tab_sb[mc][:, ns * P:(ns + 1) * P],
                            identity=ident[:])
                agin = actp.tile([P, 2048], f16, name=f"agin_{gg}", tag="agin",
                                 bufs=1)
                nc.vector.tensor_copy(out=agin[:], in_=tr_ps[:])
                nc.scalar.dma_start(
                    out=ag_in.ap()[gg * 512:(gg + 1) * 512, :]
                        .rearrange("(ns p) f -> p ns f", p=P),
                    in_=agin[:].rearrange("p (ns f) -> p ns f", f=512))
                pending_ag = gg

                # interleave layer-2 src-group blocks two groups behind their
                # AllGather so gather descgen fills Pool idle inside phase 1
                if gg >= 2:
                    l2_block(gg - 2)

            # tail: last AllGather, then the remaining layer-2 blocks
            if pending_ag is not None:
                emit_ag(pending_ag)
                pending_ag = None
            l2_block(NG - 2)
            l2_block(NG - 1)

    nc.compile()
    return nc


# ----------------------------------------------------------------------------
# Entry point
# ----------------------------------------------------------------------------

_CACHE = {}


def build_and_run(inputs, trace=False, trace_kwargs=None):
    from concourse import bass_utils

    meta, in_maps = _preprocess(**inputs)
    if meta not in _CACHE:
        _CACHE[meta] = _build(meta)
    nc = _CACHE[meta]
    res = bass_utils.run_bass_kernel_spmd(
        nc, in_maps, core_ids=list(range(NCORES)),
        trace=trace, **(trace_kwargs or {}))

    mu = np.empty((N, OUT), np.float32)
    lv = np.empty((N, OUT), np.float32)
    for c in range(NCORES):
        raw = res.results[c]["out"]            # [128, NG*2048] fp32 p-major
        blk = raw.reshape(P, NG, 4, 512).transpose(2, 0, 1, 3).reshape(
            512, NG * 512)                     # [512 ch, 2560 nodes]
        mu[c * NLOC:(c + 1) * NLOC] = blk[0:OUT, :NLOC].T
        lv[c * NLOC:(c + 1) * NLOC] = blk[OUT:2 * OUT, :NLOC].T
    return (mu, lv), res


def kernel(**inputs):
    out, _ = build_and_run(inputs, trace=False)
    return out


# revision 30
# speedup vs baseline: 1.1227x; 1.1227x over previous
"""Trainium2 Bass kernel for a 2-layer relational GraphSAGE VGAE encoder.

Contract: kernel(**inputs) takes the FULL unsharded inputs (as produced by
setup_inputs()) and returns the full (mu, logvar) tuple.

Strategy (8 NeuronCores, SPMD single NEFF):
  - Nodes block-sharded: core c owns nodes [c*2500, (c+1)*2500), padded to 2560.
  - Edges partitioned by destination-node owner into narrow 64-dst cells so
    the segment-sum one-hot matmuls emit only 64 columns per 128-edge chunk.
  - One-hot A-matrices are built ON DEVICE per chunk with a single DVE
    tensor_scalar (iota == col) * inv — no A-value DMA at all.
  - Source-row gathers are bundled 4 chunks per indirect DMA (one [128, 4]
    offset ap) to amortize SWDGE descriptor-generation overhead.
  - Layer-1 weights, the stacked layer-2 projection weight, and biases are
    SBUF-resident (loaded once). BatchNorm (eval) is folded into layer-2
    weights on the host.
  - Per node group: SAGE-1 -> relu -> both layer-2 projections (aggregated
    side 'tab' AND self side) -> transpose tab -> per-group AllGather. No
    hrelu DRAM roundtrip.
  - Layer-2 edges are split A/B by source-group owner: A = src in groups 0-3
    (gathers depend only on the first four AllGathers and overlap the last
    group's compute + AllGather), B = src in group 4 (small tail).
"""
import sys

sys.path.insert(0, "/opt/trn_rl_repo")

import numpy as np

NCORES = 8
N = 20000
E = 100000
IN = 512
HID = 512
CAT = 2560
OUT = 256
BN_EPS = 1e-5

NLOC = N // NCORES          # 2500
NPAD = 2560                 # 20 * 128, 5 * 512
NG = NPAD // 512            # 5 node groups of 512 per core
NREL = 5
P = 128
W1 = 256                    # layer-1 dst-cell width
B1 = 512 // W1              # 2 blocks per node group
W2 = 128                    # layer-2 dst-cell width
B2 = 512 // W2              # 4 blocks per node group


# ----------------------------------------------------------------------------
# Host-side preprocessing: sharding, edge chunking, weight folding
# ----------------------------------------------------------------------------

def _chunk_edges(key, ncells, src_vals, col, val, W):
    """Group edges by per-core cell, chunk each cell into 128-edge chunks.

    key: [E'] int = core * ncells + cell   (cell < ncells)
    src_vals: [E'] int32 gather row index for each edge
    col: [E'] int in [0, W) dst position within its W-wide cell
    val: [E'] f32 one-hot value (1/cnt)

    Returns: nch [ncells] shared chunk counts (max over cores, >=1),
             base [ncells] chunk base offsets, C,
             idxT [NCORES, P, C] int32, colT/invT [NCORES, P, C] f32.
    Pad slots get col=W (matches nothing in the 0..W-1 iota -> zero row).
    """
    counts = np.bincount(key, minlength=NCORES * ncells).reshape(NCORES, ncells)
    nch = np.maximum((counts + P - 1) // P, 1).max(axis=0)  # [ncells]
    base = np.concatenate([[0], np.cumsum(nch)[:-1]])
    C = int(nch.sum())

    order = np.argsort(key, kind="stable")
    ks = key[order]
    first = np.r_[True, ks[1:] != ks[:-1]]
    run_starts = np.flatnonzero(first)
    run_id = np.cumsum(first) - 1
    pos = np.arange(len(ks)) - run_starts[run_id]

    core_s = ks // ncells
    cell_s = ks % ncells
    chunk_s = base[cell_s] + pos // P
    row_s = pos % P

    idxT = np.zeros((NCORES, P, C), np.int32)
    colT = np.full((NCORES, P, C), float(W), np.float32)
    invT = np.ones((NCORES, P, C), np.float32)
    idxT[core_s, row_s, chunk_s] = src_vals[order]
    colT[core_s, row_s, chunk_s] = col[order]
    invT[core_s, row_s, chunk_s] = val[order]
    return nch, base, C, idxT, colT, invT


def _preprocess(x, edge_index, edge_attr, Wl5, Wr5, bl5,
                Wmu_l, Wmu_r, bmu, Wlv_l, Wlv_r, blv,
                gamma, beta, run_mean, run_var):
    x = np.asarray(x, np.float32)
    src = np.asarray(edge_index[0], np.int64)
    dst = np.asarray(edge_index[1], np.int64)
    rel = np.asarray(edge_attr, np.int64)

    # --- per-node degree counts ---
    cnt1 = np.bincount(rel * N + dst, minlength=NREL * N).reshape(NREL, N)
    inv1 = 1.0 / np.maximum(cnt1, 1.0)
    cnt2 = np.bincount(dst, minlength=N)
    inv2 = 1.0 / np.maximum(cnt2, 1.0)

    core = dst // NLOC
    loc = dst % NLOC

    # layer-1 cells in DEVICE consumption order: (g, k, blk within group)
    blk1 = loc // W1
    g1 = blk1 // B1
    cell1 = g1 * (NREL * B1) + rel * B1 + (blk1 % B1)
    key1 = core * (NREL * NG * B1) + cell1
    nch1, base1, C1, i1, c1, v1 = _chunk_edges(
        key1, NREL * NG * B1, src.astype(np.int32), loc % W1,
        inv1[rel, dst], W1)

    # layer-2: gather rows from the all-gathered tab tables. A = src in
    # groups 0-3 (ag_tab03, rows [srcg][core][col512]) so its gathers only
    # wait on the first four AllGathers; B = src in group 4 (ag_tab4).
    srcl = src % NLOC
    srcg = srcl // 512
    rowA = (srcg * (NCORES * 512) + (src // NLOC) * 512 + srcl % 512)
    rowB = ((src // NLOC) * 512 + srcl % 512)

    blk2 = loc // W2
    key2 = core * (NG * B2) + blk2
    mA = srcg <= 3
    mB = ~mA
    l2sets = [
        _chunk_edges(key2[mA], NG * B2, rowA[mA].astype(np.int32),
                     (loc % W2)[mA], inv2[dst[mA]], W2),
        _chunk_edges(key2[mB], NG * B2, rowB[mB].astype(np.int32),
                     (loc % W2)[mB], inv2[dst[mB]], W2),
    ]

    # --- node features ---
    xtab = x.astype(np.float16)                           # [N, 512] gather table
    xt = np.zeros((NCORES, IN, NPAD), np.float16)         # feature-major local x
    for c in range(NCORES):
        xt[c, :, :NLOC] = x[c * NLOC:(c + 1) * NLOC].T
    # partition-major: xtP[c][p, g*2048 + kc*512 + f] = xt[c][kc*128+p, g*512+f]
    xtP = np.ascontiguousarray(
        xt.reshape(NCORES, 4, P, NG, 512).transpose(0, 2, 3, 1, 4)
        .reshape(NCORES, P, NG * 2048))

    # --- weight folding (BN eval folded into layer-2 weights) ---
    f64 = np.float64
    s = np.asarray(gamma, f64) / np.sqrt(np.asarray(run_var, f64) + BN_EPS)
    t = np.asarray(beta, f64) - np.asarray(run_mean, f64) * s

    # partition-major weightT: w[k][p, kc*512 + j] = W^T[k][kc*128+p, j]
    def _pmaj_w(W5):
        wt = np.asarray(W5, np.float32).transpose(0, 2, 1).astype(np.float16)
        wt = np.ascontiguousarray(
            wt.reshape(NREL, 4, P, HID).transpose(0, 2, 1, 3)
            .reshape(NREL, P, 4 * HID))
        # resident stacked [128, 5*2048]: col = k*2048 + kc*512 + j
        return np.ascontiguousarray(
            wt.transpose(1, 0, 2).reshape(P, NREL * 4 * HID))

    wlt5 = _pmaj_w(Wl5)
    wrt5 = _pmaj_w(Wr5)

    Wtab = np.concatenate([np.asarray(Wmu_l, f64), np.asarray(Wlv_l, f64)], 0)
    Wself = np.concatenate([np.asarray(Wmu_r, f64), np.asarray(Wlv_r, f64)], 0)
    Wall = np.concatenate([Wtab * s[None, :], Wself * s[None, :]], 0)  # [1024, 2560]
    # partition-major: wallt[p, r*1024 + j] = Wall.T[r*128+p, j]
    wallt = np.ascontiguousarray(
        Wall.T.astype(np.float16).reshape(20, P, 1024).transpose(1, 0, 2)
        .reshape(P, 20 * 1024))

    tW = (Wtab @ t).astype(np.float32)                                  # [512]
    bself = (Wself @ t + np.concatenate(
        [np.asarray(bmu, f64), np.asarray(blv, f64)])).astype(np.float32)

    # bias tiles, laid out [128, n] so a column is a per-partition scalar
    blb = np.ascontiguousarray(
        np.asarray(bl5, np.float32).reshape(NREL * 4, P).T)   # [128, 20]
    twb = np.ascontiguousarray(tW.reshape(4, P).T)            # [128, 4]
    bsb = np.ascontiguousarray(bself.reshape(4, P).T)         # [128, 4]

    iota = np.broadcast_to(
        np.arange(W1, dtype=np.float16), (P, W1)).copy()      # [128, 256]

    # concatenated chunk tables [128, C1+sum(Cg)]
    idxT = np.concatenate([i1] + [s[3] for s in l2sets], axis=2)
    colT = np.concatenate([c1] + [s[4] for s in l2sets], axis=2)
    invT = np.concatenate([v1] + [s[5] for s in l2sets], axis=2)

    meta = (tuple(nch1), tuple(base1), C1,
            tuple(tuple(s[0]) for s in l2sets),
            tuple(tuple(s[1]) for s in l2sets),
            tuple(s[2] for s in l2sets))
    in_maps = []
    for c in range(NCORES):
        in_maps.append({
            "xtab": xtab, "xt": xtP[c],
            "idx": idxT[c], "colv": colT[c], "invv": invT[c],
            "wlt5": wlt5, "wrt5": wrt5, "wallt": wallt,
            "blb": blb, "twb": twb, "bsb": bsb, "iota": iota,
        })
    return meta, in_maps


# ----------------------------------------------------------------------------
# Device kernel
# ----------------------------------------------------------------------------

def _build(meta):
    import concourse.bacc as bacc
    import concourse.bass as bass
    import concourse.tile as tile
    import concourse.mybir as mybir
    from concourse.masks import make_identity

    (nch1, base1, C1, nch2s, base2s, C2s) = meta
    nch1 = np.asarray(nch1).reshape(NG, NREL, B1)
    base1 = np.asarray(base1).reshape(NG, NREL, B1)
    nch2 = [np.asarray(n).reshape(NG, B2) for n in nch2s]
    base2 = [np.asarray(b).reshape(NG, B2) for b in base2s]
    cbase2 = np.concatenate([[0], np.cumsum(C2s)[:-1]]) + C1  # per-set offset
    CT = C1 + int(np.sum(C2s))
    assert len(C2s) == 2

    f16, f32, i32 = mybir.dt.float16, mybir.dt.float32, mybir.dt.int32

    nc = bacc.Bacc("TRN2", target_bir_lowering=False, debug=False,
                   num_devices=NCORES)

    xtab_t = nc.dram_tensor("xtab", [N, IN], f16, kind="ExternalInput")
    xt_t = nc.dram_tensor("xt", [P, NG * 2048], f16, kind="ExternalInput")
    idx_t = nc.dram_tensor("idx", [P, CT], i32, kind="ExternalInput")
    col_t = nc.dram_tensor("colv", [P, CT], f32, kind="ExternalInput")
    inv_t = nc.dram_tensor("invv", [P, CT], f32, kind="ExternalInput")
    wlt5_t = nc.dram_tensor("wlt5", [P, NREL * 2048], f16, kind="ExternalInput")
    wrt5_t = nc.dram_tensor("wrt5", [P, NREL * 2048], f16, kind="ExternalInput")
    wallt_t = nc.dram_tensor("wallt", [P, 20 * 1024], f16, kind="ExternalInput")
    blb_t = nc.dram_tensor("blb", [P, NREL * 4], f32, kind="ExternalInput")
    twb_t = nc.dram_tensor("twb", [P, 4], f32, kind="ExternalInput")
    bsb_t = nc.dram_tensor("bsb", [P, 4], f32, kind="ExternalInput")
    iota_t = nc.dram_tensor("iota", [P, W1], f16, kind="ExternalInput")
    out_t = nc.dram_tensor("out", [P, NG * 2048], f32, kind="ExternalOutput")

    warm_t = nc.dram_tensor("warm", [P, 512], f16, kind="Internal")
    ag_in = nc.dram_tensor("ag_in", [NPAD, 512], f16, kind="Internal")
    hrelu = nc.dram_tensor("hrelu", [P, NG * 20 * 512], f16, kind="Internal")
    ag_tab03 = nc.dram_tensor("ag_tab03", [4 * NCORES * 512, 512], f16,
                              kind="Internal", addr_space="Shared")
    ag_tab4 = nc.dram_tensor("ag_tab4", [NCORES * 512, 512], f16,
                             kind="Internal", addr_space="Shared")

    with tile.TileContext(nc) as tc:
        with (
            tc.tile_pool(name="constp", bufs=1) as constp,
            tc.tile_pool(name="resp", bufs=1) as resp,
            tc.tile_pool(name="iop", bufs=3) as iop,
            tc.tile_pool(name="avp", bufs=8) as avp,
            tc.tile_pool(name="actp", bufs=2) as actp,
            tc.tile_pool(name="psum", bufs=2, space="PSUM") as pp,
        ):
            # ---- constants / resident tiles ----
            ident = constp.tile([P, P], f16, name="ident", tag="ident")
            make_identity(nc, ident[:])
            blb_sb = constp.tile([P, NREL * 4], f32, name="blb_sb", tag="blb")
            nc.sync.dma_start(out=blb_sb[:], in_=blb_t.ap())
            twb_sb = constp.tile([P, 4], f32, name="twb_sb", tag="twb")
            nc.sync.dma_start(out=twb_sb[:], in_=twb_t.ap())
            bsb_sb = constp.tile([P, 4], f32, name="bsb_sb", tag="bsb")
            nc.sync.dma_start(out=bsb_sb[:], in_=bsb_t.ap())
            iota_sb = constp.tile([P, W1], f16, name="iota_sb", tag="iota")
            nc.sync.dma_start(out=iota_sb[:], in_=iota_t.ap())

            # chunk tables via gpsimd's own SWDGE queue so the first gathers
            # don't wait on sync-ring semaphore-lane chains
            idx_sb = resp.tile([P, CT], i32, name="idx_sb", tag="idx")
            nc.gpsimd.dma_start(out=idx_sb[:], in_=idx_t.ap())
            col_sb = resp.tile([P, CT], f32, name="col_sb", tag="colv")
            nc.gpsimd.dma_start(out=col_sb[:], in_=col_t.ap())
            inv_sb = resp.tile([P, CT], f32, name="inv_sb", tag="invv")
            nc.gpsimd.dma_start(out=inv_sb[:], in_=inv_t.ap())

            # resident weights
            wl5_sb = resp.tile([P, NREL * 2048], f16, name="wl5_sb", tag="wl5")
            nc.scalar.dma_start(out=wl5_sb[:], in_=wlt5_t.ap())
            wr5_sb = resp.tile([P, NREL * 2048], f16, name="wr5_sb", tag="wr5")
            nc.scalar.dma_start(out=wr5_sb[:], in_=wrt5_t.ap())
            wall_sb = resp.tile([P, 20 * 1024], f16, name="wall_sb", tag="wall")

            def wall_chunk(r, mc):
                return wall_sb[:, r * 1024 + mc * P:r * 1024 + (mc + 1) * P]

            # self-side projections + layer-2 A-partials, resident f16
            sf_sb = resp.tile([P, NG * 2048], f16, name="sf_sb", tag="sf")
            aA_sb = resp.tile([P, NG * 2048], f16, name="aA_sb", tag="aA")

            # ---- PE warm-up: a few throwaway matmuls while DMAs fill ----
            wu = constp.tile([P, 512], f16, name="wu", tag="wu")
            nc.vector.memset(wu[:], 0.0)
            wu_ps = pp.tile([P, 2048], f32, space="PSUM", name="wu_ps", tag="big")
            for i in range(32):
                nc.tensor.matmul(out=wu_ps[:, 0:512], lhsT=wu[:, 0:P],
                                 rhs=wu[:], start=(i == 0), stop=(i == 31))
            nc.vector.tensor_copy(out=wu[:], in_=wu_ps[:, 0:512])
            nc.sync.dma_start(out=warm_t.ap(), in_=wu[:])

            # ---- gather + on-device one-hot machinery --------------------
            def agg_cell(ps, b, W, nchunks, cbase, table, pref):
                """Accumulate one W-dst cell into ps[:, cc*512 + b*W ...]."""
                l2 = pref.startswith("l2")
                tiles = []
                for ci in range(nchunks):
                    gi = cbase + ci
                    gt = iop.tile([P, 512], f16, name=f"g_{pref}_{gi}",
                                  tag="gth2" if l2 else "gth",
                                  bufs=6 if l2 else 20)
                    nc.gpsimd.indirect_dma_start(
                        out=gt[:], out_offset=None,
                        in_=table.ap(),
                        in_offset=bass.IndirectOffsetOnAxis(
                            ap=idx_sb[:, gi:gi + 1], axis=0))
                    av = avp.tile([P, W1], f16, name=f"av_{pref}_{gi}",
                                  tag="av")
                    nc.vector.tensor_scalar(
                        out=av[:, 0:W], in0=iota_sb[:, 0:W],
                        scalar1=col_sb[:, gi:gi + 1],
                        scalar2=inv_sb[:, gi:gi + 1],
                        op0=mybir.AluOpType.is_equal, op1=mybir.AluOpType.mult)
                    tiles.append((gt, av))
                for cc in range(4):
                    for ci, (gt, av) in enumerate(tiles):
                        nc.tensor.matmul(
                            out=ps[:, cc * 512 + b * W:cc * 512 + (b + 1) * W],
                            lhsT=gt[:, cc * P:(cc + 1) * P],
                            rhs=av[:, 0:W],
                            start=(ci == 0), stop=(ci == nchunks - 1))

            # ====== Phase 1, g-outer: SAGE layer 1 -> both projections
            # -> transpose -> per-group AllGather (collectives overlap P1).
            def emit_ag(g):
                dst = (ag_tab4 if g == 4 else ag_tab03)
                lo = 0 if g == 4 else g * NCORES * 512
                nc.gpsimd.collective_compute(
                    "AllGather", mybir.AluOpType.bypass,
                    replica_groups=[list(range(NCORES))],
                    ins=[ag_in.ap()[g * 512:(g + 1) * 512, :]],
                    outs=[dst.ap()[lo:lo + NCORES * 512, :]])

            def l2_block(si):
                """Aggregate layer-2 set si: 0 = A (src groups 0-3, into the
                resident aA_sb partial), 1 = B (src group 4) + combine/out."""
                table = ag_tab03 if si == 0 else ag_tab4
                for dg in range(NG):
                    ps = pp.tile([P, 2048], f32, space="PSUM",
                                 name=f"m2_{si}_{dg}", tag="big")
                    for b in range(B2):
                        agg_cell(ps, b, W2, int(nch2[si][dg, b]),
                                 int(cbase2[si]) + int(base2[si][dg, b]),
                                 table, f"l2{si}")
                    osl = slice(dg * 2048, (dg + 1) * 2048)
                    if si == 0:
                        nc.vector.tensor_copy(out=aA_sb[:, osl], in_=ps[:])
                    else:
                        ob = actp.tile([P, 2048], f32, name=f"out_{dg}",
                                       tag="outsb", bufs=1)
                        for mc in range(4):
                            sl = slice(mc * 512, (mc + 1) * 512)
                            gsl = slice(dg * 2048 + mc * 512,
                                        dg * 2048 + (mc + 1) * 512)
                            nc.vector.tensor_tensor(
                                out=ob[:, sl], in0=ps[:, sl],
                                in1=aA_sb[:, gsl], op=mybir.AluOpType.add)
                            nc.vector.tensor_tensor(
                                out=ob[:, sl], in0=ob[:, sl],
                                in1=sf_sb[:, gsl], op=mybir.AluOpType.add)
                        nc.scalar.dma_start(
                            out=out_t.ap()[:, dg * 2048:(dg + 1) * 2048],
                            in_=ob[:])

            pending_ag = None
            for gg in range(NG):
                xg = actp.tile([P, 2048], f16, name=f"xg_{gg}", tag="xg",
                               bufs=1)
                nc.scalar.dma_start(
                    out=xg[:], in_=xt_t.ap()[:, gg * 2048:(gg + 1) * 2048])
                if gg == 0:
                    nc.scalar.dma_start(out=wall_sb[:], in_=wallt_t.ap())
                rts = []
                for k in range(NREL):
                    # --- aggregation: mean_k^T for this node group ---
                    mean_ps = pp.tile([P, 2048], f32, space="PSUM",
                                      name=f"agg_{gg}_{k}", tag="big")
                    for b in range(B1):
                        agg_cell(mean_ps, b, W1, int(nch1[gg, k, b]),
                                 int(base1[gg, k, b]), xtab_t, "l1")
                    mean_sb = []
                    for cc in range(4):
                        m = actp.tile([P, 512], f16, name=f"mean_{gg}_{k}_{cc}",
                                      tag=f"mean{cc}")
                        nc.vector.tensor_copy(
                            out=m[:], in_=mean_ps[:, cc * 512:(cc + 1) * 512])
                        mean_sb.append(m)

                    # --- dense: h = relu(Wl@mean + Wr@x + b) ---
                    h_ps = pp.tile([P, 2048], f32, space="PSUM",
                                   name=f"h_{gg}_{k}", tag="big")
                    for kc in range(4):
                        base = k * 2048 + kc * 512
                        for mc in range(4):
                            nc.tensor.matmul(
                                out=h_ps[:, mc * 512:(mc + 1) * 512],
                                lhsT=wl5_sb[:, base + mc * P:base + (mc + 1) * P],
                                rhs=mean_sb[kc][:], start=(kc == 0), stop=False)
                    for kc in range(4):
                        base = k * 2048 + kc * 512
                        for mc in range(4):
                            nc.tensor.matmul(
                                out=h_ps[:, mc * 512:(mc + 1) * 512],
                                lhsT=wr5_sb[:, base + mc * P:base + (mc + 1) * P],
                                rhs=xg[:, kc * 512:(kc + 1) * 512],
                                start=False, stop=(kc == 3))
                    rt = actp.tile([P, 2048], f16, name=f"relu_{gg}_{k}",
                                   tag=f"rt{k}", bufs=1)
                    for mc in range(4):
                        nc.vector.tensor_scalar(
                            out=rt[:, mc * 512:(mc + 1) * 512],
                            in0=h_ps[:, mc * 512:(mc + 1) * 512],
                            scalar1=blb_sb[:, k * 4 + mc:k * 4 + mc + 1],
                            scalar2=0.0,
                            op0=mybir.AluOpType.add, op1=mybir.AluOpType.max)
                    nc.scalar.dma_start(
                        out=hrelu.ap()[:, (gg * 20 + k * 4) * 512:
                                       (gg * 20 + k * 4 + 4) * 512],
                        in_=rt[:])
                    rts.append(rt)

                if pending_ag is not None:
                    emit_ag(pending_ag)
                    pending_ag = None

                # --- aggregated-side (tab) projections only; self side is
                # deferred to the tail where it covers the layer-2 descgen
                tab_sb = []
                p_ps = pp.tile([P, 2048], f32, space="PSUM",
                               name=f"proj_{gg}", tag="big")
                for mc4 in range(4):
                    o = p_ps[:, mc4 * 512:(mc4 + 1) * 512]
                    for r in range(20):
                        nc.tensor.matmul(
                            out=o, lhsT=wall_chunk(r, mc4),
                            rhs=rts[r // 4][:, (r % 4) * 512:(r % 4 + 1) * 512],
                            start=(r == 0), stop=(r == 19))
                for mc4 in range(4):
                    tab = actp.tile([P, 512], f16, name=f"tab_{gg}_{mc4}",
                                    tag=f"tab{mc4}", bufs=1)
                    nc.vector.tensor_scalar(
                        out=tab[:], in0=p_ps[:, mc4 * 512:(mc4 + 1) * 512],
                        scalar1=twb_sb[:, mc4:mc4 + 1],
                        scalar2=None, op0=mybir.AluOpType.add)
                    tab_sb.append(tab)

                tr_ps = pp.tile([P, 2048], f16, space="PSUM",
                                name=f"tr_{gg}", tag="big")
                for ns in range(4):
                    for mc in range(4):
                        nc.tensor.transpose(
                            out=tr_ps[:, (ns * 4 + mc) * P:(ns * 4 + mc + 1) * P],
                            in_=tab_sb[mc][:, ns * P:(ns + 1) * P],
                            identity=ident[:])
                agin = actp.tile([P, 2048], f16, name=f"agin_{gg}", tag="agin",
                                 bufs=1)
                nc.vector.tensor_copy(out=agin[:], in_=tr_ps[:])
                nc.scalar.dma_start(
                    out=ag_in.ap()[gg * 512:(gg + 1) * 512, :]
                        .rearrange("(ns p) f -> p ns f", p=P),
                    in_=agin[:].rearrange("p (ns f) -> p ns f", f=512))
                pending_ag = gg

            # last AllGather goes out before any layer-2 gather descgen
            if pending_ag is not None:
                emit_ag(pending_ag)
                pending_ag = None

            # ====== Phase 3a: self-side projections (reload hrelu). Pure
            # PE/DVE work that covers AllGather 4 and the layer-2 descgen.
            for gg in range(NG):
                rts = []
                for k in range(NREL):
                    rt = actp.tile([P, 2048], f16, name=f"hrB_{gg}_{k}",
                                   tag=f"rt{k}", bufs=1)
                    nc.sync.dma_start(
                        out=rt[:],
                        in_=hrelu.ap()[:, (gg * 20 + k * 4) * 512:
                                       (gg * 20 + k * 4 + 4) * 512])
                    rts.append(rt)
                s_ps = pp.tile([P, 2048], f32, space="PSUM",
                               name=f"self_{gg}", tag="big")
                for mc4 in range(4):
                    o = s_ps[:, mc4 * 512:(mc4 + 1) * 512]
                    for r in range(20):
                        nc.tensor.matmul(
                            out=o, lhsT=wall_chunk(r, 4 + mc4),
                            rhs=rts[r // 4][:, (r % 4) * 512:(r % 4 + 1) * 512],
                            start=(r == 0), stop=(r == 19))
                for mc4 in range(4):
                    nc.vector.tensor_scalar(
                        out=sf_sb[:, gg * 2048 + mc4 * 512:
                                  gg * 2048 + (mc4 + 1) * 512],
                        in0=s_ps[:, mc4 * 512:(mc4 + 1) * 512],
                        scalar1=bsb_sb[:, mc4:mc4 + 1],
                        scalar2=None, op0=mybir.AluOpType.add)

            # ====== Phase 3b: layer-2 aggregation A then B + combine/out
            l2_block(0)
            l2_block(1)

    nc.compile()
    return nc


# ----------------------------------------------------------------------------
# Entry point
# ----------------------------------------------------------------------------

_CACHE = {}


def build_and_run(inputs, trace=False, trace_kwargs=None):
    from concourse import bass_utils

    meta, in_maps = _preprocess(**inputs)
    if meta not in _CACHE:
        _CACHE[meta] = _build(meta)
    nc = _CACHE[meta]
    res = bass_utils.run_bass_kernel_spmd(
        nc, in_maps, core_ids=list(range(NCORES)),
        trace=trace, **(trace_kwargs or {}))

    mu = np.empty((N, OUT), np.float32)
    lv = np.empty((N, OUT), np.float32)
    for c in range(NCORES):
        raw = res.results[c]["out"]            # [128, NG*2048] fp32 p-major
        blk = raw.reshape(P, NG, 4, 512).transpose(2, 0, 1, 3).reshape(
            512, NG * 512)                     # [512 ch, 2560 nodes]
        mu[c * NLOC:(c + 1) * NLOC] = blk[0:OUT, :NLOC].T
        lv[c * NLOC:(c + 1) * NLOC] = blk[OUT:2 * OUT, :NLOC].T
    return (mu, lv), res


def kernel(**inputs):
    out, _ = build_and_run(inputs, trace=False)
    return out


# revision 34
# speedup vs baseline: 1.2874x; 1.1468x over previous
"""Trainium2 Bass kernel for a 2-layer relational GraphSAGE VGAE encoder.

Contract: kernel(**inputs) takes the FULL unsharded inputs (as produced by
setup_inputs()) and returns the full (mu, logvar) tuple.

Strategy (8 NeuronCores, SPMD single NEFF):
  - Nodes block-sharded: core c owns nodes [c*2500, (c+1)*2500), padded to 2560.
  - Edges partitioned by destination-node owner into narrow 64-dst cells so
    the segment-sum one-hot matmuls emit only 64 columns per 128-edge chunk.
  - One-hot A-matrices are built ON DEVICE per chunk with a single DVE
    tensor_scalar (iota == col) * inv — no A-value DMA at all.
  - Source-row gathers are bundled 4 chunks per indirect DMA (one [128, 4]
    offset ap) to amortize SWDGE descriptor-generation overhead.
  - Layer-1 weights, the stacked layer-2 projection weight, and biases are
    SBUF-resident (loaded once). BatchNorm (eval) is folded into layer-2
    weights on the host.
  - Per node group: SAGE-1 -> relu -> both layer-2 projections (aggregated
    side 'tab' AND self side) -> transpose tab -> per-group AllGather. No
    hrelu DRAM roundtrip.
  - Layer-2 edges are split A/B by source-group owner: A = src in groups 0-3
    (gathers depend only on the first four AllGathers and overlap the last
    group's compute + AllGather), B = src in group 4 (small tail).
"""
import sys

sys.path.insert(0, "/opt/trn_rl_repo")

import numpy as np

NCORES = 8
N = 20000
E = 100000
IN = 512
HID = 512
CAT = 2560
OUT = 256
BN_EPS = 1e-5

NLOC = N // NCORES          # 2500
NPAD = 2560                 # 20 * 128, 5 * 512
NG = NPAD // 512            # 5 node groups of 512 per core
NREL = 5
P = 128
W1 = 256                    # layer-1 dst-cell width
B1 = 512 // W1              # 2 blocks per node group
W2 = 128                    # layer-2 dst-cell width
B2 = 512 // W2              # 4 blocks per node group


# ----------------------------------------------------------------------------
# Host-side preprocessing: sharding, edge chunking, weight folding
# ----------------------------------------------------------------------------

def _chunk_edges(key, ncells, src_vals, col, val, W):
    """Group edges by per-core cell, chunk each cell into 128-edge chunks.

    key: [E'] int = core * ncells + cell   (cell < ncells)
    src_vals: [E'] int32 gather row index for each edge
    col: [E'] int in [0, W) dst position within its W-wide cell
    val: [E'] f32 one-hot value (1/cnt)

    Returns: nch [ncells] shared chunk counts (max over cores, >=1),
             base [ncells] chunk base offsets, C,
             idxT [NCORES, P, C] int32, colT/invT [NCORES, P, C] f32.
    Pad slots get col=W (matches nothing in the 0..W-1 iota -> zero row).
    """
    counts = np.bincount(key, minlength=NCORES * ncells).reshape(NCORES, ncells)
    nch = np.maximum((counts + P - 1) // P, 1).max(axis=0)  # [ncells]
    base = np.concatenate([[0], np.cumsum(nch)[:-1]])
    C = int(nch.sum())

    order = np.argsort(key, kind="stable")
    ks = key[order]
    first = np.r_[True, ks[1:] != ks[:-1]]
    run_starts = np.flatnonzero(first)
    run_id = np.cumsum(first) - 1
    pos = np.arange(len(ks)) - run_starts[run_id]

    core_s = ks // ncells
    cell_s = ks % ncells
    chunk_s = base[cell_s] + pos // P
    row_s = pos % P

    idxT = np.zeros((NCORES, P, C), np.int32)
    colT = np.full((NCORES, P, C), float(W), np.float32)
    invT = np.ones((NCORES, P, C), np.float32)
    idxT[core_s, row_s, chunk_s] = src_vals[order]
    colT[core_s, row_s, chunk_s] = col[order]
    invT[core_s, row_s, chunk_s] = val[order]
    return nch, base, C, idxT, colT, invT


def _preprocess(x, edge_index, edge_attr, Wl5, Wr5, bl5,
                Wmu_l, Wmu_r, bmu, Wlv_l, Wlv_r, blv,
                gamma, beta, run_mean, run_var):
    x = np.asarray(x, np.float32)
    src = np.asarray(edge_index[0], np.int64)
    dst = np.asarray(edge_index[1], np.int64)
    rel = np.asarray(edge_attr, np.int64)

    # --- per-node degree counts ---
    cnt1 = np.bincount(rel * N + dst, minlength=NREL * N).reshape(NREL, N)
    inv1 = 1.0 / np.maximum(cnt1, 1.0)
    cnt2 = np.bincount(dst, minlength=N)
    inv2 = 1.0 / np.maximum(cnt2, 1.0)

    core = dst // NLOC
    loc = dst % NLOC

    # layer-1 cells in DEVICE consumption order: (g, k, blk within group)
    blk1 = loc // W1
    g1 = blk1 // B1
    cell1 = g1 * (NREL * B1) + rel * B1 + (blk1 % B1)
    key1 = core * (NREL * NG * B1) + cell1
    nch1, base1, C1, i1, c1, v1 = _chunk_edges(
        key1, NREL * NG * B1, src.astype(np.int32), loc % W1,
        inv1[rel, dst], W1)

    # layer-2: gather rows from the all-gathered tab tables. A = src in
    # groups 0-3 (ag_tab03, rows [srcg][core][col512]) so its gathers only
    # wait on the first four AllGathers; B = src in group 4 (ag_tab4).
    srcl = src % NLOC
    srcg = srcl // 512
    rowA = (srcg * (NCORES * 512) + (src // NLOC) * 512 + srcl % 512)
    rowB = ((src // NLOC) * 512 + srcl % 512)

    blk2 = loc // W2
    key2 = core * (NG * B2) + blk2
    mA = srcg <= 3
    mB = ~mA
    l2sets = [
        _chunk_edges(key2[mA], NG * B2, rowA[mA].astype(np.int32),
                     (loc % W2)[mA], inv2[dst[mA]], W2),
        _chunk_edges(key2[mB], NG * B2, rowB[mB].astype(np.int32),
                     (loc % W2)[mB], inv2[dst[mB]], W2),
    ]

    # --- node features ---
    xtab = x.astype(np.float16)                           # [N, 512] gather table
    xt = np.zeros((NCORES, IN, NPAD), np.float16)         # feature-major local x
    for c in range(NCORES):
        xt[c, :, :NLOC] = x[c * NLOC:(c + 1) * NLOC].T
    # partition-major: xtP[c][p, g*2048 + kc*512 + f] = xt[c][kc*128+p, g*512+f]
    xtP = np.ascontiguousarray(
        xt.reshape(NCORES, 4, P, NG, 512).transpose(0, 2, 3, 1, 4)
        .reshape(NCORES, P, NG * 2048))

    # --- weight folding (BN eval folded into layer-2 weights) ---
    f64 = np.float64
    s = np.asarray(gamma, f64) / np.sqrt(np.asarray(run_var, f64) + BN_EPS)
    t = np.asarray(beta, f64) - np.asarray(run_mean, f64) * s

    # partition-major weightT: w[k][p, kc*512 + j] = W^T[k][kc*128+p, j]
    def _pmaj_w(W5):
        wt = np.asarray(W5, np.float32).transpose(0, 2, 1).astype(np.float16)
        wt = np.ascontiguousarray(
            wt.reshape(NREL, 4, P, HID).transpose(0, 2, 1, 3)
            .reshape(NREL, P, 4 * HID))
        # resident stacked [128, 5*2048]: col = k*2048 + kc*512 + j
        return np.ascontiguousarray(
            wt.transpose(1, 0, 2).reshape(P, NREL * 4 * HID))

    wlt5 = _pmaj_w(Wl5)
    wrt5 = _pmaj_w(Wr5)

    Wtab = np.concatenate([np.asarray(Wmu_l, f64), np.asarray(Wlv_l, f64)], 0)
    Wself = np.concatenate([np.asarray(Wmu_r, f64), np.asarray(Wlv_r, f64)], 0)
    Wall = np.concatenate([Wtab * s[None, :], Wself * s[None, :]], 0)  # [1024, 2560]
    # partition-major: wallt[p, r*1024 + j] = Wall.T[r*128+p, j]
    wallt = np.ascontiguousarray(
        Wall.T.astype(np.float16).reshape(20, P, 1024).transpose(1, 0, 2)
        .reshape(P, 20 * 1024))

    tW = (Wtab @ t).astype(np.float32)                                  # [512]
    bself = (Wself @ t + np.concatenate(
        [np.asarray(bmu, f64), np.asarray(blv, f64)])).astype(np.float32)

    # bias tiles, laid out [128, n] so a column is a per-partition scalar
    blb = np.ascontiguousarray(
        np.asarray(bl5, np.float32).reshape(NREL * 4, P).T)   # [128, 20]
    twb = np.ascontiguousarray(tW.reshape(4, P).T)            # [128, 4]
    bsb = np.ascontiguousarray(bself.reshape(4, P).T)         # [128, 4]

    iota = np.broadcast_to(
        np.arange(W1, dtype=np.float16), (P, W1)).copy()      # [128, 256]

    # concatenated chunk tables [128, C1+sum(Cg)]
    idxT = np.concatenate([i1] + [s[3] for s in l2sets], axis=2)
    colT = np.concatenate([c1] + [s[4] for s in l2sets], axis=2)
    invT = np.concatenate([v1] + [s[5] for s in l2sets], axis=2)

    meta = (tuple(nch1), tuple(base1), C1,
            tuple(tuple(s[0]) for s in l2sets),
            tuple(tuple(s[1]) for s in l2sets),
            tuple(s[2] for s in l2sets))
    in_maps = []
    for c in range(NCORES):
        in_maps.append({
            "xtab": xtab, "xt": xtP[c],
            "idx": idxT[c], "colv": colT[c], "invv": invT[c],
            "wlt5": wlt5, "wrt5": wrt5, "wallt": wallt,
            "blb": blb, "twb": twb, "bsb": bsb, "iota": iota,
        })
    return meta, in_maps


# ----------------------------------------------------------------------------
# Device kernel
# ----------------------------------------------------------------------------

def _build(meta):
    import concourse.bacc as bacc
    import concourse.bass as bass
    import concourse.tile as tile
    import concourse.mybir as mybir
    from concourse.masks import make_identity

    (nch1, base1, C1, nch2s, base2s, C2s) = meta
    nch1 = np.asarray(nch1).reshape(NG, NREL, B1)
    base1 = np.asarray(base1).reshape(NG, NREL, B1)
    nch2 = [np.asarray(n).reshape(NG, B2) for n in nch2s]
    base2 = [np.asarray(b).reshape(NG, B2) for b in base2s]
    cbase2 = np.concatenate([[0], np.cumsum(C2s)[:-1]]) + C1  # per-set offset
    CT = C1 + int(np.sum(C2s))
    assert len(C2s) == 2

    f16, f32, i32 = mybir.dt.float16, mybir.dt.float32, mybir.dt.int32

    nc = bacc.Bacc("TRN2", target_bir_lowering=False, debug=False,
                   num_devices=NCORES)

    xtab_t = nc.dram_tensor("xtab", [N, IN], f16, kind="ExternalInput")
    xt_t = nc.dram_tensor("xt", [P, NG * 2048], f16, kind="ExternalInput")
    idx_t = nc.dram_tensor("idx", [P, CT], i32, kind="ExternalInput")
    col_t = nc.dram_tensor("colv", [P, CT], f32, kind="ExternalInput")
    inv_t = nc.dram_tensor("invv", [P, CT], f32, kind="ExternalInput")
    wlt5_t = nc.dram_tensor("wlt5", [P, NREL * 2048], f16, kind="ExternalInput")
    wrt5_t = nc.dram_tensor("wrt5", [P, NREL * 2048], f16, kind="ExternalInput")
    wallt_t = nc.dram_tensor("wallt", [P, 20 * 1024], f16, kind="ExternalInput")
    blb_t = nc.dram_tensor("blb", [P, NREL * 4], f32, kind="ExternalInput")
    twb_t = nc.dram_tensor("twb", [P, 4], f32, kind="ExternalInput")
    bsb_t = nc.dram_tensor("bsb", [P, 4], f32, kind="ExternalInput")
    iota_t = nc.dram_tensor("iota", [P, W1], f16, kind="ExternalInput")
    out_t = nc.dram_tensor("out", [P, NG * 2048], f32, kind="ExternalOutput")

    warm_t = nc.dram_tensor("warm", [P, 512], f16, kind="Internal")
    ag_in = nc.dram_tensor("ag_in", [NPAD, 512], f16, kind="Internal")
    hrelu = nc.dram_tensor("hrelu", [P, NG * 20 * 512], f16, kind="Internal")
    ag_tab03 = nc.dram_tensor("ag_tab03", [4 * NCORES * 512, 512], f16,
                              kind="Internal", addr_space="Shared")
    ag_tab4 = nc.dram_tensor("ag_tab4", [NCORES * 512, 512], f16,
                             kind="Internal", addr_space="Shared")

    with tile.TileContext(nc) as tc:
        with (
            tc.tile_pool(name="constp", bufs=1) as constp,
            tc.tile_pool(name="resp", bufs=1) as resp,
            tc.tile_pool(name="iop", bufs=3) as iop,
            tc.tile_pool(name="avp", bufs=8) as avp,
            tc.tile_pool(name="actp", bufs=2) as actp,
            tc.tile_pool(name="psum", bufs=2, space="PSUM") as pp,
        ):
            # ---- constants / resident tiles ----
            ident = constp.tile([P, P], f16, name="ident", tag="ident")
            make_identity(nc, ident[:])
            blb_sb = constp.tile([P, NREL * 4], f32, name="blb_sb", tag="blb")
            nc.sync.dma_start(out=blb_sb[:], in_=blb_t.ap())
            twb_sb = constp.tile([P, 4], f32, name="twb_sb", tag="twb")
            nc.sync.dma_start(out=twb_sb[:], in_=twb_t.ap())
            bsb_sb = constp.tile([P, 4], f32, name="bsb_sb", tag="bsb")
            nc.sync.dma_start(out=bsb_sb[:], in_=bsb_t.ap())
            iota_sb = constp.tile([P, W1], f16, name="iota_sb", tag="iota")
            nc.sync.dma_start(out=iota_sb[:], in_=iota_t.ap())

            # chunk tables via gpsimd's own SWDGE queue so the first gathers
            # don't wait on sync-ring semaphore-lane chains
            idx_sb = resp.tile([P, CT], i32, name="idx_sb", tag="idx")
            nc.gpsimd.dma_start(out=idx_sb[:], in_=idx_t.ap())
            col_sb = resp.tile([P, CT], f32, name="col_sb", tag="colv")
            nc.gpsimd.dma_start(out=col_sb[:], in_=col_t.ap())
            inv_sb = resp.tile([P, CT], f32, name="inv_sb", tag="invv")
            nc.gpsimd.dma_start(out=inv_sb[:], in_=inv_t.ap())

            # resident weights
            wl5_sb = resp.tile([P, NREL * 2048], f16, name="wl5_sb", tag="wl5")
            nc.scalar.dma_start(out=wl5_sb[:], in_=wlt5_t.ap())
            wr5_sb = resp.tile([P, NREL * 2048], f16, name="wr5_sb", tag="wr5")
            nc.scalar.dma_start(out=wr5_sb[:], in_=wrt5_t.ap())
            wall_sb = resp.tile([P, 20 * 1024], f16, name="wall_sb", tag="wall")

            def wall_chunk(r, mc):
                return wall_sb[:, r * 1024 + mc * P:r * 1024 + (mc + 1) * P]

            # self-side projections + layer-2 A-partials, resident f16
            sf_sb = resp.tile([P, NG * 2048], f16, name="sf_sb", tag="sf")
            aA_sb = resp.tile([P, NG * 2048], f16, name="aA_sb", tag="aA")

            # ---- PE warm-up: a few throwaway matmuls while DMAs fill ----
            wu = constp.tile([P, 512], f16, name="wu", tag="wu")
            nc.vector.memset(wu[:], 0.0)
            wu_ps = pp.tile([P, 2048], f32, space="PSUM", name="wu_ps", tag="big")
            for i in range(32):
                nc.tensor.matmul(out=wu_ps[:, 0:512], lhsT=wu[:, 0:P],
                                 rhs=wu[:], start=(i == 0), stop=(i == 31))
            nc.vector.tensor_copy(out=wu[:], in_=wu_ps[:, 0:512])
            nc.sync.dma_start(out=warm_t.ap(), in_=wu[:])

            # ---- gather + on-device one-hot machinery --------------------
            def agg_cell(ps, b, W, nchunks, cbase, table, pref):
                """Accumulate one W-dst cell into ps[:, cc*512 + b*W ...]."""
                l2 = pref.startswith("l2")
                tiles = []
                for ci in range(nchunks):
                    gi = cbase + ci
                    gt = iop.tile([P, 512], f16, name=f"g_{pref}_{gi}",
                                  tag="gth2" if l2 else "gth",
                                  bufs=10 if l2 else 20)
                    nc.gpsimd.indirect_dma_start(
                        out=gt[:], out_offset=None,
                        in_=table.ap(),
                        in_offset=bass.IndirectOffsetOnAxis(
                            ap=idx_sb[:, gi:gi + 1], axis=0))
                    av = avp.tile([P, W1], f16, name=f"av_{pref}_{gi}",
                                  tag="av")
                    nc.vector.tensor_scalar(
                        out=av[:, 0:W], in0=iota_sb[:, 0:W],
                        scalar1=col_sb[:, gi:gi + 1],
                        scalar2=inv_sb[:, gi:gi + 1],
                        op0=mybir.AluOpType.is_equal, op1=mybir.AluOpType.mult)
                    tiles.append((gt, av))
                for cc in range(4):
                    for ci, (gt, av) in enumerate(tiles):
                        nc.tensor.matmul(
                            out=ps[:, cc * 512 + b * W:cc * 512 + (b + 1) * W],
                            lhsT=gt[:, cc * P:(cc + 1) * P],
                            rhs=av[:, 0:W],
                            start=(ci == 0), stop=(ci == nchunks - 1))

            # ====== Phase 1, g-outer: SAGE layer 1 -> both projections
            # -> transpose -> per-group AllGather (collectives overlap P1).
            def emit_ag(g):
                dst = (ag_tab4 if g == 4 else ag_tab03)
                lo = 0 if g == 4 else g * NCORES * 512
                nc.gpsimd.collective_compute(
                    "AllGather", mybir.AluOpType.bypass,
                    replica_groups=[list(range(NCORES))],
                    ins=[ag_in.ap()[g * 512:(g + 1) * 512, :]],
                    outs=[dst.ap()[lo:lo + NCORES * 512, :]])

            def l2_dg(si, dg):
                """Aggregate layer-2 set si for dst group dg: si 0 = A (src
                groups 0-3, into the resident aA_sb partial), si 1 = B (src
                group 4) + combine/out."""
                table = ag_tab03 if si == 0 else ag_tab4
                ps = pp.tile([P, 2048], f32, space="PSUM",
                             name=f"m2_{si}_{dg}", tag="big")
                for b in range(B2):
                    agg_cell(ps, b, W2, int(nch2[si][dg, b]),
                             int(cbase2[si]) + int(base2[si][dg, b]),
                             table, f"l2{si}")
                if si == 0:
                    nc.vector.tensor_copy(
                        out=aA_sb[:, dg * 2048:(dg + 1) * 2048], in_=ps[:])
                else:
                    ob = actp.tile([P, 2048], f32, name=f"out_{dg}",
                                   tag="outsb", bufs=1)
                    for mc in range(4):
                        sl = slice(mc * 512, (mc + 1) * 512)
                        gsl = slice(dg * 2048 + mc * 512,
                                    dg * 2048 + (mc + 1) * 512)
                        nc.vector.tensor_tensor(
                            out=ob[:, sl], in0=ps[:, sl],
                            in1=aA_sb[:, gsl], op=mybir.AluOpType.add)
                        nc.vector.tensor_tensor(
                            out=ob[:, sl], in0=ob[:, sl],
                            in1=sf_sb[:, gsl], op=mybir.AluOpType.add)
                    nc.scalar.dma_start(
                        out=out_t.ap()[:, dg * 2048:(dg + 1) * 2048],
                        in_=ob[:])

            pending_ag = None
            for gg in range(NG):
                xg = actp.tile([P, 2048], f16, name=f"xg_{gg}", tag="xg",
                               bufs=1)
                nc.scalar.dma_start(
                    out=xg[:], in_=xt_t.ap()[:, gg * 2048:(gg + 1) * 2048])
                if gg == 0:
                    nc.scalar.dma_start(out=wall_sb[:], in_=wallt_t.ap())
                rts = []
                for k in range(NREL):
                    # --- aggregation: mean_k^T for this node group ---
                    mean_ps = pp.tile([P, 2048], f32, space="PSUM",
                                      name=f"agg_{gg}_{k}", tag="big")
                    for b in range(B1):
                        agg_cell(mean_ps, b, W1, int(nch1[gg, k, b]),
                                 int(base1[gg, k, b]), xtab_t, "l1")
                    mean_sb = []
                    for cc in range(4):
                        m = actp.tile([P, 512], f16, name=f"mean_{gg}_{k}_{cc}",
                                      tag=f"mean{cc}")
                        nc.vector.tensor_copy(
                            out=m[:], in_=mean_ps[:, cc * 512:(cc + 1) * 512])
                        mean_sb.append(m)

                    # --- dense: h = relu(Wl@mean + Wr@x + b) ---
                    h_ps = pp.tile([P, 2048], f32, space="PSUM",
                                   name=f"h_{gg}_{k}", tag="big")
                    for kc in range(4):
                        base = k * 2048 + kc * 512
                        for mc in range(4):
                            nc.tensor.matmul(
                                out=h_ps[:, mc * 512:(mc + 1) * 512],
                                lhsT=wl5_sb[:, base + mc * P:base + (mc + 1) * P],
                                rhs=mean_sb[kc][:], start=(kc == 0), stop=False)
                    for kc in range(4):
                        base = k * 2048 + kc * 512
                        for mc in range(4):
                            nc.tensor.matmul(
                                out=h_ps[:, mc * 512:(mc + 1) * 512],
                                lhsT=wr5_sb[:, base + mc * P:base + (mc + 1) * P],
                                rhs=xg[:, kc * 512:(kc + 1) * 512],
                                start=False, stop=(kc == 3))
                    rt = actp.tile([P, 2048], f16, name=f"relu_{gg}_{k}",
                                   tag=f"rt{k}", bufs=1)
                    for mc in range(4):
                        nc.vector.tensor_scalar(
                            out=rt[:, mc * 512:(mc + 1) * 512],
                            in0=h_ps[:, mc * 512:(mc + 1) * 512],
                            scalar1=blb_sb[:, k * 4 + mc:k * 4 + mc + 1],
                            scalar2=0.0,
                            op0=mybir.AluOpType.add, op1=mybir.AluOpType.max)
                    nc.scalar.dma_start(
                        out=hrelu.ap()[:, (gg * 20 + k * 4) * 512:
                                       (gg * 20 + k * 4 + 4) * 512],
                        in_=rt[:])
                    rts.append(rt)

                if pending_ag is not None:
                    emit_ag(pending_ag)
                    pending_ag = None

                # --- aggregated-side (tab) projections only; self side is
                # deferred to the tail where it covers the layer-2 descgen
                tab_sb = []
                p_ps = pp.tile([P, 2048], f32, space="PSUM",
                               name=f"proj_{gg}", tag="big")
                for mc4 in range(4):
                    o = p_ps[:, mc4 * 512:(mc4 + 1) * 512]
                    for r in range(20):
                        nc.tensor.matmul(
                            out=o, lhsT=wall_chunk(r, mc4),
                            rhs=rts[r // 4][:, (r % 4) * 512:(r % 4 + 1) * 512],
                            start=(r == 0), stop=(r == 19))
                for mc4 in range(4):
                    tab = actp.tile([P, 512], f16, name=f"tab_{gg}_{mc4}",
                                    tag=f"tab{mc4}", bufs=1)
                    nc.vector.tensor_scalar(
                        out=tab[:], in0=p_ps[:, mc4 * 512:(mc4 + 1) * 512],
                        scalar1=twb_sb[:, mc4:mc4 + 1],
                        scalar2=None, op0=mybir.AluOpType.add)
                    tab_sb.append(tab)

                tr_ps = pp.tile([P, 2048], f16, space="PSUM",
                                name=f"tr_{gg}", tag="big")
                for ns in range(4):
                    for mc in range(4):
                        nc.tensor.transpose(
                            out=tr_ps[:, (ns * 4 + mc) * P:(ns * 4 + mc + 1) * P],
                            in_=tab_sb[mc][:, ns * P:(ns + 1) * P],
                            identity=ident[:])
                agin = actp.tile([P, 2048], f16, name=f"agin_{gg}", tag="agin",
                                 bufs=1)
                nc.vector.tensor_copy(out=agin[:], in_=tr_ps[:])
                nc.scalar.dma_start(
                    out=ag_in.ap()[gg * 512:(gg + 1) * 512, :]
                        .rearrange("(ns p) f -> p ns f", p=P),
                    in_=agin[:].rearrange("p (ns f) -> p ns f", f=512))
                pending_ag = gg

            # last AllGather goes out before any layer-2 gather descgen
            if pending_ag is not None:
                emit_ag(pending_ag)
                pending_ag = None

            # ====== Phase 3: per dst-group, layer-2 A aggregation first
            # (keeps Pool descgen streaming), then the self-side projection
            # for one node group as PE cover; B (src group 4) last.
            for gg in range(NG):
                l2_dg(0, gg)
                rts = []
                for k in range(NREL):
                    rt = actp.tile([P, 2048], f16, name=f"hrB_{gg}_{k}",
                                   tag=f"rt{k}", bufs=1)
                    nc.sync.dma_start(
                        out=rt[:],
                        in_=hrelu.ap()[:, (gg * 20 + k * 4) * 512:
                                       (gg * 20 + k * 4 + 4) * 512])
                    rts.append(rt)
                s_ps = pp.tile([P, 2048], f32, space="PSUM",
                               name=f"self_{gg}", tag="big")
                for mc4 in range(4):
                    o = s_ps[:, mc4 * 512:(mc4 + 1) * 512]
                    for r in range(20):
                        nc.tensor.matmul(
                            out=o, lhsT=wall_chunk(r, 4 + mc4),
                            rhs=rts[r // 4][:, (r % 4) * 512:(r % 4 + 1) * 512],
                            start=(r == 0), stop=(r == 19))
                for mc4 in range(4):
                    nc.vector.tensor_scalar(
                        out=sf_sb[:, gg * 2048 + mc4 * 512:
                                  gg * 2048 + (mc4 + 1) * 512],
                        in0=s_ps[:, mc4 * 512:(mc4 + 1) * 512],
                        scalar1=bsb_sb[:, mc4:mc4 + 1],
                        scalar2=None, op0=mybir.AluOpType.add)

            # ====== Phase 3b: layer-2 B aggregation + combine/out
            for dg in range(NG):
                l2_dg(1, dg)

    nc.compile()
    return nc


# ----------------------------------------------------------------------------
# Entry point
# ----------------------------------------------------------------------------

_CACHE = {}


def build_and_run(inputs, trace=False, trace_kwargs=None):
    from concourse import bass_utils

    meta, in_maps = _preprocess(**inputs)
    if meta not in _CACHE:
        _CACHE[meta] = _build(meta)
    nc = _CACHE[meta]
    res = bass_utils.run_bass_kernel_spmd(
        nc, in_maps, core_ids=list(range(NCORES)),
        trace=trace, **(trace_kwargs or {}))

    mu = np.empty((N, OUT), np.float32)
    lv = np.empty((N, OUT), np.float32)
    for c in range(NCORES):
        raw = res.results[c]["out"]            # [128, NG*2048] fp32 p-major
        blk = raw.reshape(P, NG, 4, 512).transpose(2, 0, 1, 3).reshape(
            512, NG * 512)                     # [512 ch, 2560 nodes]
        mu[c * NLOC:(c + 1) * NLOC] = blk[0:OUT, :NLOC].T
        lv[c * NLOC:(c + 1) * NLOC] = blk[OUT:2 * OUT, :NLOC].T
    return (mu, lv), res


def kernel(**inputs):
    out, _ = build_and_run(inputs, trace=False)
    return out
